# revision 37
# baseline (speedup 1.0000x reference)
"""AdditiveAttention Trainium2 kernel (8 NeuronCores, SPMD, no collectives).

reference:
    q = queries @ Wq               (B,Q,H)
    k = keys @ Wk                  (B,K,H)
    scores[b,q,k] = sum_h wv[h] * tanh(q[b,q,h] + k[b,k,h])
    masked = where(arange(K) < valid_lens[b], scores, 0.0)
    attn = softmax(masked, -1)      # masked cols contribute e^0 = 1
    out = attn @ values             (B,Q,D)

Sharding: core c = (b, q_half) -> computes out[b, qh*128:(qh+1)*128, :].
Each core owns 128 queries x full K of one batch. Purely data-parallel,
no cross-core reduction.

Per-core structure (h=H=128 on partitions for the score stage):
  - kpT[h, k], qpT[h, q] via PE matmuls (bf16 in, f32 accum)
  - per q: ONE ScalarE activation computes tanh(kpT + qpT[:, q]) fused
    (per-partition bias), output bf16 [128, KE]
  - per q: PE matmuls with a 32-wide "sliding window" stationary operand
    (wv at column q%32, zeros elsewhere) accumulate that q's scores into
    row q (partition base 32*(q//32)) of a PSUM tile -> dense scores[q,k]
  - per 64-row half: mask multiply (masked logits -> 0), exp on ScalarE
    with accum_out giving the softmax denominator for free, PE transpose
    of E -> E_T; the first half overlaps the second half's tanh stream
  - attn@V matmuls (bf16); k >= KE tail handled by all-ones stationary
    operand (exp(0) == 1 there); normalize with per-partition 1/Z

KE = ceil(max(valid_lens)/128)*128 <= K: columns >= KE are masked in every
batch, so tanh/exp work shrinks to KE columns.
"""

import sys

sys.path.insert(0, "/opt/trn_rl_repo")

from contextlib import ExitStack

import numpy as np
import ml_dtypes

import concourse.bass as bass
import concourse.mybir as mybir
import concourse.tile as tile
from concourse import bacc
from concourse.bass_utils import run_bass_kernel_spmd
from concourse.masks import make_identity

B, Q, K, D, H = 4, 256, 1024, 512, 128
QS = Q // 2  # queries per core
N_CORES = 8
F32 = mybir.dt.float32
BF16 = mybir.dt.bfloat16
BF16_NP = np.dtype(ml_dtypes.bfloat16)
WU_MM = 9  # PE warmup matmuls (~3.8us cold, under the input-DMA shadow)


def build_graph(KE: int) -> bass.Bass:
    assert KE % 128 == 0 and 128 <= KE <= K
    DC = D // 128  # contraction chunks for the projections
    # n-chunks (<=512) of the score/exp free axis
    k_chunks = [(s, min(512, KE - s)) for s in range(0, KE, 512)]
    KC128 = KE // 128
    VC = K // 128
    HQ = QS // 2  # epilogue half

    H0 = 96  # rows finished early (hidden under the tanh stream)
    H1 = QS - H0

    nc = bacc.Bacc("TRN2", target_bir_lowering=False, debug=False)

    # all inputs arrive host-packed as the exact SBUF image ([128, N],
    # contiguous per partition) so every DMA runs at max descriptor size.
    # kT is additionally packed k-chunk-major so each k-chunk half is a
    # contiguous column range (split across the two HWDGE rings).
    qT_d = nc.declare_dram_parameter("qT", [128, DC * QS], BF16, isOutput=False)
    kT_d = nc.declare_dram_parameter("kT", [128, DC * KE], BF16, isOutput=False)
    v_d = nc.declare_dram_parameter("v", [128, VC * D], BF16, isOutput=False)
    wq_d = nc.declare_dram_parameter("wq", [128, DC * H], BF16, isOutput=False)
    wk_d = nc.declare_dram_parameter("wk", [128, DC * H], BF16, isOutput=False)
    # [128, 128] bf16 sliding windows: col 30 = wv (even q%32), col 64+31 = wv
    wvwin_d = nc.declare_dram_parameter("wvwin", [H, 128], BF16, isOutput=False)
    mask_d = nc.declare_dram_parameter("mask", [H0, KE], BF16, isOutput=False)
    out_d = nc.declare_dram_parameter("out", [QS, D], F32, isOutput=True)

    with tile.TileContext(nc) as tc, ExitStack() as ctx:
        const = ctx.enter_context(tc.tile_pool(name="const", bufs=1))
        work = ctx.enter_context(tc.tile_pool(name="work", bufs=1))
        tq_pool = ctx.enter_context(tc.tile_pool(name="tq", bufs=3))
        xa_pool = ctx.enter_context(tc.tile_pool(name="xa", bufs=3))
        pp = ctx.enter_context(tc.tile_pool(name="pp", bufs=1, space="PSUM"))
        scp = ctx.enter_context(tc.tile_pool(name="scp", bufs=1, space="PSUM"))
        tpp = ctx.enter_context(tc.tile_pool(name="tpp", bufs=2, space="PSUM"))
        pop = ctx.enter_context(tc.tile_pool(name="pop", bufs=1, space="PSUM"))

        # ---- load inputs (few big DMAs, spread over both HWDGE rings) ----
        kT_sb = const.tile([128, DC * KE], BF16, tag="kT")
        v_sb = const.tile([128, VC * D], BF16, tag="v")
        qT_sb = const.tile([128, DC * QS], BF16, tag="qT")
        wq_sb = const.tile([128, DC * H], BF16, tag="wq")
        wk_sb = const.tile([128, DC * H], BF16, tag="wk")
        wvwin_sb = const.tile([H, 128], BF16, tag="wvwin")
        mask_sb = const.tile([H0, KE], BF16, tag="mask")
        # kT split on the first k-chunk boundary across both HWDGE rings;
        # chunk-major host packing makes both halves contiguous columns
        kcut = DC * k_chunks[0][1]
        nc.sync.dma_start(kT_sb[:, :kcut], kT_d[:, :kcut])
        if kcut < DC * KE:
            nc.scalar.dma_start(kT_sb[:, kcut:], kT_d[:, kcut:])
        nc.sync.dma_start(wk_sb[:], wk_d[:, :])
        nc.scalar.dma_start(qT_sb[:], qT_d[:, :])
        nc.scalar.dma_start(wq_sb[:], wq_d[:, :])
        nc.sync.dma_start(wvwin_sb[:], wvwin_d[:, :])
        nc.scalar.dma_start(v_sb[:], v_d[:, :])
        nc.scalar.dma_start(mask_sb[:], mask_d[:, :])

        def kT_ci(ci, i):
            """d-chunk i of k-chunk ci, as packed: [base_ci + i*w, +w)."""
            base = DC * sum(w for _, w in k_chunks[:ci])
            w = k_chunks[ci][1]
            return kT_sb[:, base + i * w : base + (i + 1) * w]

        def v_c(i):
            return v_sb[:, i * D : (i + 1) * D]

        # ---- PE warmup burst (HAM un-throttle) under the DMA shadow ----
        wu_in = const.tile([128, 512], BF16, tag="wu_in")
        nc.gpsimd.memset(wu_in[:], 0.0)
        wu_ps = pop.tile([128, 512], F32, tag="po", name="wu_ps")
        for i in range(WU_MM):
            nc.tensor.matmul(
                wu_ps[:], wu_in[:, :128], wu_in[:], start=True, stop=True
            )

        # ---- projections: kpT[h, k] first (gates the tanh stream) ----
        kp_ps = pp.tile([H, KE], F32, tag="kp_ps")
        kp_sb = work.tile([H, KE], F32, tag="kp_sb")
        for ci, (s, w) in enumerate(k_chunks):
            for i in range(DC):
                nc.tensor.matmul(
                    kp_ps[:, s : s + w],
                    wk_sb[:, i * H : (i + 1) * H],
                    kT_ci(ci, i),
                    start=(i == 0),
                    stop=(i == DC - 1),
                )
            nc.vector.tensor_copy(kp_sb[:, s : s + w], kp_ps[:, s : s + w])
        qp_ps = scp.tile([H, QS], F32, tag="sc_ps", name="qp_ps")
        for i in range(DC):
            nc.tensor.matmul(
                qp_ps[:],
                wq_sb[:, i * H : (i + 1) * H],
                qT_sb[:, i * QS : (i + 1) * QS],
                start=(i == 0),
                stop=(i == DC - 1),
            )
        qp_sb = work.tile([H, QS], F32, tag="qp_sb")
        nc.vector.tensor_copy(qp_sb[:], qp_ps[:])

        ident = const.tile([128, 128], BF16, tag="ident")
        make_identity(nc, ident[:])
        ones_sb = const.tile([128, 128], BF16, tag="ones")
        nc.gpsimd.memset(ones_sb[:], 1.0)

        et_sb = work.tile([128, KC128 * 128], BF16, tag="et_sb")
        out_sb = work.tile([QS, D], F32, tag="out_sb")

        def scores_mm(q, tq_ap, sc_h):
            """score scatter matmuls for one query row from its tanh slice."""
            g, r = divmod(q if q < H0 else q - H0, 32)
            off = (30 - r) if r % 2 == 0 else (64 + 31 - r)
            win = wvwin_sb[:, off : off + 32]
            for s, w in k_chunks:
                nc.tensor.matmul(
                    sc_h[g * 32 : (g + 1) * 32, s : s + w],
                    win,
                    tq_ap[:, s : s + w],
                    start=(r == 0),
                    stop=(r == 31),
                    tile_position=(0, g * 32),
                )

        def q_pair_block(q0, sc_of):
            """Broadcast-add on DVE (per-partition scalar), pure tanh on
            ScalarE over a fused pair of query rows (amortizes the ~224-cycle
            ACT per-instruction overhead), then the score matmuls."""
            xa = xa_pool.tile([H, 2 * KE], F32, tag="xa", name="xa")
            nc.vector.tensor_scalar_add(xa[:, :KE], kp_sb[:], qp_sb[:, q0 : q0 + 1])
            nc.vector.tensor_scalar_add(
                xa[:, KE:], kp_sb[:], qp_sb[:, q0 + 1 : q0 + 2]
            )
            tq = tq_pool.tile([H, 2 * KE], BF16, tag="tq", name="tq")
            nc.scalar.activation(tq[:], xa[:], mybir.ActivationFunctionType.Tanh)
            scores_mm(q0, tq[:, :KE], sc_of(q0))
            scores_mm(q0 + 1, tq[:, KE:], sc_of(q0 + 1))

        def epilogue_part(h, sc_h, po_h, r0, nr):
            """mask + exp + transpose + attn@V + normalize + store for query
            rows [r0, r0+nr).

            All tiles here live on partitions 0:nr (engines cannot shift
            partitions); the q-offset reappears as a column offset in et_sb
            and as the DRAM row offset of the output DMA.
            """
            msk_h = work.tile([nr, KE], F32, tag=f"msk{h}", name=f"msk{h}")
            e_h = work.tile([nr, KE], BF16, tag=f"e{h}", name=f"e{h}")
            z_h = work.tile([nr, 2], F32, tag=f"z{h}", name=f"z{h}")
            # mask+exp in k-halves so exp(half0) overlaps mask(half1)
            ecut = k_chunks[0][1]
            for ei, (es, ew) in enumerate([(0, ecut), (ecut, KE - ecut)]):
                if ew <= 0:
                    continue
                nc.vector.tensor_mul(
                    msk_h[:, es : es + ew],
                    sc_h[:, es : es + ew],
                    mask_sb[:nr, es : es + ew],
                )
                nc.scalar.activation(
                    e_h[:, es : es + ew],
                    msk_h[:, es : es + ew],
                    mybir.ActivationFunctionType.Exp,
                    accum_out=z_h[:, ei : ei + 1],
                )
            # transposes in two waves over two PSUM banks; each wave's evac
            # (DVE) overlaps the other wave's transposes (PE)
            W0 = KC128 // 2
            waves = [(0, W0), (W0, KC128 - W0)]
            for c0, ncw in waves:
                tp = tpp.tile([128, (KC128 - W0) * nr], BF16, tag="tp", name="tp")
                for j in range(ncw):
                    nc.tensor.transpose(
                        tp[:, j * nr : (j + 1) * nr],
                        e_h[:, (c0 + j) * 128 : (c0 + j + 1) * 128],
                        ident[:nr, :nr],
                    )
                nc.vector.tensor_copy(
                    et_sb[:]
                    .rearrange("p (c n) -> p c n", c=KC128)[
                        :, c0 : c0 + ncw, r0 : r0 + nr
                    ],
                    tp[:, : ncw * nr].rearrange("p (c n) -> p c n", c=ncw),
                )
            # attn @ V (tail k-chunks beyond KE use ones: exp(0) = 1)
            for c in range(VC):
                lhsT = (
                    et_sb[:, c * 128 + r0 : c * 128 + r0 + nr]
                    if c < KC128
                    else ones_sb[:, :nr]
                )
                nc.tensor.matmul(
                    po_h[:],
                    lhsT,
                    v_c(c),
                    start=(c == 0),
                    stop=(c == VC - 1),
                )
            # normalize + store (partition-aligned: po_h/z_h live on 0:nr)
            z2 = work.tile([nr, 1], F32, tag=f"z2_{h}", name=f"z2_{h}")
            nc.vector.tensor_scalar_add(z2[:], z_h[:, 0:1], float(K - KE))
            if KE > ecut:
                nc.vector.tensor_add(z2[:], z2[:], z_h[:, 1:2])
            rz = work.tile([nr, 1], F32, tag=f"rz{h}", name=f"rz{h}")
            nc.vector.reciprocal(rz[:], z2[:])
            nc.vector.tensor_scalar_mul(out_sb[:nr, :], po_h[:], rz[:])
            nc.sync.dma_start(out_d[r0 : r0 + nr, :], out_sb[:nr, :])

        # per-part PSUM score tiles -> disjoint banks, so the early epilogue
        # can read its scores while PE still accumulates the rest (same-bank
        # PE-W + engine-R is a hardware race)
        sc_h0 = scp.tile([H0, KE], F32, tag="sc_ps", name="sc_h0")
        for q in range(0, H0, 2):
            q_pair_block(q, lambda q_: sc_h0)
        po_h0 = pop.tile([H0, D], F32, tag="po", name="po_h0")
        epilogue_part(0, sc_h0, po_h0, 0, H0)
        sc_h1 = pp.tile([H1, KE], F32, tag="kp_ps", name="sc_h1")
        for q in range(H0, QS, 2):
            q_pair_block(q, lambda q_: sc_h1)
        po_h1 = pp.tile([H1, D], F32, tag="kp_ps", name="po_h1")
        epilogue_part(1, sc_h1, po_h1, H0, H1)

    nc.compile()
    return nc


_GRAPH_CACHE: dict[int, bass.Bass] = {}
_LAST_RESULTS = None


def _get_graph(KE: int) -> bass.Bass:
    if KE not in _GRAPH_CACHE:
        _GRAPH_CACHE[KE] = build_graph(KE)
    return _GRAPH_CACHE[KE]


def _sbuf_pack(mat_T):
    """[R*128, N] -> [128, R*N]: SBUF image with d-chunks along columns."""
    R = mat_T.shape[0] // 128
    return np.ascontiguousarray(
        mat_T.reshape(R, 128, -1).transpose(1, 0, 2).reshape(128, -1)
    )


def make_in_maps(queries, keys, values, Wq, Wk, wv, valid_lens, KE):
    wvwin = np.zeros((H, 128), BF16_NP)
    wvwin[:, 30] = wv.astype(BF16_NP)
    wvwin[:, 64 + 31] = wv.astype(BF16_NP)
    col = np.arange(KE)
    # k-chunk-major packing for kT: [128, sum_ci DC*w_ci]
    k_chunks = [(s, min(512, KE - s)) for s in range(0, KE, 512)]
    in_maps = []
    for c in range(N_CORES):
        b, qh = divmod(c, 2)
        mask_row = (col < int(valid_lens[b])).astype(np.float32)
        kT = keys[b, :KE, :].T.astype(BF16_NP)  # [D, KE]
        kT_packed = np.concatenate(
            [_sbuf_pack(kT[:, s : s + w]) for s, w in k_chunks], axis=1
        )
        in_maps.append(
            {
                "qT": _sbuf_pack(
                    queries[b, qh * QS : (qh + 1) * QS, :].T.astype(BF16_NP)
                ),
                "kT": np.ascontiguousarray(kT_packed),
                "v": _sbuf_pack(values[b].astype(BF16_NP)),
                "wq": _sbuf_pack(Wq.astype(BF16_NP)),
                "wk": _sbuf_pack(Wk.astype(BF16_NP)),
                "wvwin": wvwin,
                "mask": np.ascontiguousarray(
                    np.broadcast_to(mask_row, (96, KE)).astype(BF16_NP)
                ),
            }
        )
    return in_maps


def kernel(queries, keys, values, Wq, Wk, wv, valid_lens, **run_kwargs):
    queries = np.asarray(queries, np.float32)
    keys = np.asarray(keys, np.float32)
    values = np.asarray(values, np.float32)
    Wq = np.asarray(Wq, np.float32)
    Wk = np.asarray(Wk, np.float32)
    wv = np.asarray(wv, np.float32)
    valid_lens = np.asarray(valid_lens, np.int32)

    KE = int(-(-int(valid_lens.max()) // 128) * 128)
    KE = max(128, min(K, KE))

    nc = _get_graph(KE)
    in_maps = make_in_maps(queries, keys, values, Wq, Wk, wv, valid_lens, KE)
    res = run_bass_kernel_spmd(
        nc, in_maps, core_ids=list(range(N_CORES)), **run_kwargs
    )
    global _LAST_RESULTS
    _LAST_RESULTS = res
    out = np.empty((B, Q, D), np.float32)
    for c in range(N_CORES):
        b, qh = divmod(c, 2)
        out[b, qh * QS : (qh + 1) * QS, :] = res.results[c]["out"]
    return out


# revision 39
# speedup vs baseline: 1.2133x; 1.2133x over previous
"""AdditiveAttention Trainium2 kernel (8 NeuronCores, SPMD, no collectives).

reference:
    q = queries @ Wq               (B,Q,H)
    k = keys @ Wk                  (B,K,H)
    scores[b,q,k] = sum_h wv[h] * tanh(q[b,q,h] + k[b,k,h])
    masked = where(arange(K) < valid_lens[b], scores, 0.0)
    attn = softmax(masked, -1)      # masked cols contribute e^0 = 1
    out = attn @ values             (B,Q,D)

Sharding: core c = (b, q_half) -> computes out[b, qh*128:(qh+1)*128, :].
Each core owns 128 queries x full K of one batch. Purely data-parallel,
no cross-core reduction.

Per-core structure (h=H=128 on partitions for the score stage):
  - kpT[h, k], qpT[h, q] via PE matmuls (bf16 in, f32 accum)
  - per q: ONE ScalarE activation computes tanh(kpT + qpT[:, q]) fused
    (per-partition bias), output bf16 [128, KE]
  - per q: PE matmuls with a 32-wide "sliding window" stationary operand
    (wv at column q%32, zeros elsewhere) accumulate that q's scores into
    row q (partition base 32*(q//32)) of a PSUM tile -> dense scores[q,k]
  - per 64-row half: mask multiply (masked logits -> 0), exp on ScalarE
    with accum_out giving the softmax denominator for free, PE transpose
    of E -> E_T; the first half overlaps the second half's tanh stream
  - attn@V matmuls (bf16); k >= KE tail handled by all-ones stationary
    operand (exp(0) == 1 there); normalize with per-partition 1/Z

KE = ceil(max(valid_lens)/128)*128 <= K: columns >= KE are masked in every
batch, so tanh/exp work shrinks to KE columns.
"""

import sys

sys.path.insert(0, "/opt/trn_rl_repo")

from contextlib import ExitStack

import numpy as np
import ml_dtypes

import concourse.bass as bass
import concourse.mybir as mybir
import concourse.tile as tile
from concourse import bacc
from concourse.bass_utils import run_bass_kernel_spmd
from concourse.masks import make_identity

B, Q, K, D, H = 4, 256, 1024, 512, 128
QS = Q // 2  # queries per core
N_CORES = 8
F32 = mybir.dt.float32
BF16 = mybir.dt.bfloat16
BF16_NP = np.dtype(ml_dtypes.bfloat16)
WU_MM = 9  # PE warmup matmuls (~3.8us cold, under the input-DMA shadow)


def build_graph(KE: int) -> bass.Bass:
    assert KE % 128 == 0 and 128 <= KE <= K
    DC = D // 128  # contraction chunks for the projections
    # n-chunks (<=512) of the score/exp free axis
    k_chunks = [(s, min(512, KE - s)) for s in range(0, KE, 512)]
    KC128 = KE // 128
    VC = K // 128
    HQ = QS // 2  # epilogue half

    H0 = 96  # rows finished early (hidden under the tanh stream)
    H1 = QS - H0

    nc = bacc.Bacc("TRN2", target_bir_lowering=False, debug=False)

    # all inputs arrive host-packed as the exact SBUF image ([128, N],
    # contiguous per partition) so every DMA runs at max descriptor size.
    # kT is additionally packed k-chunk-major so each k-chunk half is a
    # contiguous column range (split across the two HWDGE rings).
    qT_d = nc.declare_dram_parameter("qT", [128, DC * QS], BF16, isOutput=False)
    kT_d = nc.declare_dram_parameter("kT", [128, DC * KE], BF16, isOutput=False)
    v_d = nc.declare_dram_parameter("v", [128, VC * D], BF16, isOutput=False)
    wq_d = nc.declare_dram_parameter("wq", [128, DC * H], BF16, isOutput=False)
    wk_d = nc.declare_dram_parameter("wk", [128, DC * H], BF16, isOutput=False)
    # [128, 128] bf16 sliding windows: col 30 = wv (even q%32), col 64+31 = wv
    wvwin_d = nc.declare_dram_parameter("wvwin", [H, 128], BF16, isOutput=False)
    mask_d = nc.declare_dram_parameter("mask", [H0, KE], BF16, isOutput=False)
    out_d = nc.declare_dram_parameter("out", [QS, D], F32, isOutput=True)

    with tile.TileContext(nc) as tc, ExitStack() as ctx:
        const = ctx.enter_context(tc.tile_pool(name="const", bufs=1))
        work = ctx.enter_context(tc.tile_pool(name="work", bufs=1))
        tq_pool = ctx.enter_context(tc.tile_pool(name="tq", bufs=3))
        xa_pool = ctx.enter_context(tc.tile_pool(name="xa", bufs=3))
        pp = ctx.enter_context(tc.tile_pool(name="pp", bufs=1, space="PSUM"))
        scp = ctx.enter_context(tc.tile_pool(name="scp", bufs=1, space="PSUM"))
        tpp = ctx.enter_context(tc.tile_pool(name="tpp", bufs=2, space="PSUM"))
        pop = ctx.enter_context(tc.tile_pool(name="pop", bufs=1, space="PSUM"))

        # ---- load inputs (few big DMAs, spread over both HWDGE rings) ----
        kT_sb = const.tile([128, DC * KE], BF16, tag="kT")
        v_sb = const.tile([128, VC * D], BF16, tag="v")
        qT_sb = const.tile([128, DC * QS], BF16, tag="qT")
        wq_sb = const.tile([128, DC * H], BF16, tag="wq")
        wk_sb = const.tile([128, DC * H], BF16, tag="wk")
        wvwin_sb = const.tile([H, 128], BF16, tag="wvwin")
        mask_sb = const.tile([H0, KE], BF16, tag="mask")
        # kT split on the first k-chunk boundary across both HWDGE rings;
        # chunk-major host packing makes both halves contiguous columns
        kcut = DC * k_chunks[0][1]
        nc.sync.dma_start(kT_sb[:, :kcut], kT_d[:, :kcut])
        if kcut < DC * KE:
            nc.scalar.dma_start(kT_sb[:, kcut:], kT_d[:, kcut:])
        nc.sync.dma_start(wk_sb[:], wk_d[:, :])
        nc.scalar.dma_start(qT_sb[:], qT_d[:, :])
        nc.scalar.dma_start(wq_sb[:], wq_d[:, :])
        nc.sync.dma_start(wvwin_sb[:], wvwin_d[:, :])
        nc.scalar.dma_start(v_sb[:], v_d[:, :])
        nc.scalar.dma_start(mask_sb[:], mask_d[:, :])

        def kT_ci(ci, i):
            """d-chunk i of k-chunk ci, as packed: [base_ci + i*w, +w)."""
            base = DC * sum(w for _, w in k_chunks[:ci])
            w = k_chunks[ci][1]
            return kT_sb[:, base + i * w : base + (i + 1) * w]

        def v_c(i):
            return v_sb[:, i * D : (i + 1) * D]

        # ---- PE warmup burst (HAM un-throttle) under the DMA shadow ----
        wu_in = const.tile([128, 512], BF16, tag="wu_in")
        nc.gpsimd.memset(wu_in[:], 0.0)
        wu_ps = pop.tile([128, 512], F32, tag="po", name="wu_ps")
        for i in range(WU_MM):
            nc.tensor.matmul(
                wu_ps[:], wu_in[:, :128], wu_in[:], start=True, stop=True
            )

        # ---- projections: kpT[h, k] first (gates the tanh stream) ----
        kp_ps = pp.tile([H, KE], F32, tag="kp_ps")
        kp_sb = work.tile([H, KE], F32, tag="kp_sb")
        for ci, (s, w) in enumerate(k_chunks):
            for i in range(DC):
                nc.tensor.matmul(
                    kp_ps[:, s : s + w],
                    wk_sb[:, i * H : (i + 1) * H],
                    kT_ci(ci, i),
                    start=(i == 0),
                    stop=(i == DC - 1),
                )
            nc.vector.tensor_copy(kp_sb[:, s : s + w], kp_ps[:, s : s + w])
        qp_ps = scp.tile([H, QS], F32, tag="sc_ps", name="qp_ps")
        for i in range(DC):
            nc.tensor.matmul(
                qp_ps[:],
                wq_sb[:, i * H : (i + 1) * H],
                qT_sb[:, i * QS : (i + 1) * QS],
                start=(i == 0),
                stop=(i == DC - 1),
            )
        qp_sb = work.tile([H, QS], F32, tag="qp_sb")
        nc.vector.tensor_copy(qp_sb[:], qp_ps[:])

        ident = const.tile([128, 128], BF16, tag="ident")
        make_identity(nc, ident[:])
        ones_sb = const.tile([128, 128], BF16, tag="ones")
        nc.gpsimd.memset(ones_sb[:], 1.0)

        et_sb = work.tile([128, KC128 * 128], BF16, tag="et_sb")
        out_sb = work.tile([QS, D], F32, tag="out_sb")

        def scores_mm(q, tq_ap, sc_h):
            """score scatter matmuls for one query row from its tanh slice."""
            g, r = divmod(q if q < H0 else q - H0, 32)
            off = (30 - r) if r % 2 == 0 else (64 + 31 - r)
            win = wvwin_sb[:, off : off + 32]
            for s, w in k_chunks:
                nc.tensor.matmul(
                    sc_h[g * 32 : (g + 1) * 32, s : s + w],
                    win,
                    tq_ap[:, s : s + w],
                    start=(r == 0),
                    stop=(r == 31),
                    tile_position=(0, g * 32),
                )

        QG = 4  # queries fused per ScalarE tanh instruction

        def q_group_block(q0, sc_of):
            """Broadcast-add on DVE (per-partition scalar), pure tanh on
            ScalarE over a fused group of query rows (amortizes the ~224-cycle
            ACT per-instruction overhead), then the score matmuls."""
            xa = xa_pool.tile([H, QG * KE], F32, tag="xa", name="xa")
            for j in range(QG):
                nc.vector.tensor_scalar_add(
                    xa[:, j * KE : (j + 1) * KE],
                    kp_sb[:],
                    qp_sb[:, q0 + j : q0 + j + 1],
                )
            tq = tq_pool.tile([H, QG * KE], BF16, tag="tq", name="tq")
            nc.scalar.activation(tq[:], xa[:], mybir.ActivationFunctionType.Tanh)
            for j in range(QG):
                scores_mm(q0 + j, tq[:, j * KE : (j + 1) * KE], sc_of(q0 + j))

        def epilogue_part(h, sc_h, po_h, r0, nr):
            """mask + exp + transpose + attn@V + normalize + store for query
            rows [r0, r0+nr).

            All tiles here live on partitions 0:nr (engines cannot shift
            partitions); the q-offset reappears as a column offset in et_sb
            and as the DRAM row offset of the output DMA.
            """
            msk_h = work.tile([nr, KE], F32, tag=f"msk{h}", name=f"msk{h}")
            e_h = work.tile([nr, KE], BF16, tag=f"e{h}", name=f"e{h}")
            z_h = work.tile([nr, 2], F32, tag=f"z{h}", name=f"z{h}")
            # mask+exp in k-halves so exp(half0) overlaps mask(half1)
            ecut = k_chunks[0][1]
            for ei, (es, ew) in enumerate([(0, ecut), (ecut, KE - ecut)]):
                if ew <= 0:
                    continue
                nc.vector.tensor_mul(
                    msk_h[:, es : es + ew],
                    sc_h[:, es : es + ew],
                    mask_sb[:nr, es : es + ew],
                )
                nc.scalar.activation(
                    e_h[:, es : es + ew],
                    msk_h[:, es : es + ew],
                    mybir.ActivationFunctionType.Exp,
                    accum_out=z_h[:, ei : ei + 1],
                )
            # transposes in two waves over two PSUM banks; each wave's evac
            # (DVE) overlaps the other wave's transposes (PE)
            W0 = KC128 // 2
            waves = [(0, W0), (W0, KC128 - W0)]
            for c0, ncw in waves:
                tp = tpp.tile([128, (KC128 - W0) * nr], BF16, tag="tp", name="tp")
                for j in range(ncw):
                    nc.tensor.transpose(
                        tp[:, j * nr : (j + 1) * nr],
                        e_h[:, (c0 + j) * 128 : (c0 + j + 1) * 128],
                        ident[:nr, :nr],
                    )
                nc.vector.tensor_copy(
                    et_sb[:]
                    .rearrange("p (c n) -> p c n", c=KC128)[
                        :, c0 : c0 + ncw, r0 : r0 + nr
                    ],
                    tp[:, : ncw * nr].rearrange("p (c n) -> p c n", c=ncw),
                )
            # attn @ V (tail k-chunks beyond KE use ones: exp(0) = 1)
            for c in range(VC):
                lhsT = (
                    et_sb[:, c * 128 + r0 : c * 128 + r0 + nr]
                    if c < KC128
                    else ones_sb[:, :nr]
                )
                nc.tensor.matmul(
                    po_h[:],
                    lhsT,
                    v_c(c),
                    start=(c == 0),
                    stop=(c == VC - 1),
                )
            # normalize + store (partition-aligned: po_h/z_h live on 0:nr)
            z2 = work.tile([nr, 1], F32, tag=f"z2_{h}", name=f"z2_{h}")
            nc.vector.tensor_scalar_add(z2[:], z_h[:, 0:1], float(K - KE))
            if KE > ecut:
                nc.vector.tensor_add(z2[:], z2[:], z_h[:, 1:2])
            rz = work.tile([nr, 1], F32, tag=f"rz{h}", name=f"rz{h}")
            nc.vector.reciprocal(rz[:], z2[:])
            nc.vector.tensor_scalar_mul(out_sb[:nr, :], po_h[:], rz[:])
            nc.sync.dma_start(out_d[r0 : r0 + nr, :], out_sb[:nr, :])

        # per-part PSUM score tiles -> disjoint banks, so the early epilogue
        # can read its scores while PE still accumulates the rest (same-bank
        # PE-W + engine-R is a hardware race)
        sc_h0 = scp.tile([H0, KE], F32, tag="sc_ps", name="sc_h0")
        for q in range(0, H0, QG):
            q_group_block(q, lambda q_: sc_h0)
        po_h0 = pop.tile([H0, D], F32, tag="po", name="po_h0")
        epilogue_part(0, sc_h0, po_h0, 0, H0)
        sc_h1 = pp.tile([H1, KE], F32, tag="kp_ps", name="sc_h1")
        for q in range(H0, QS, QG):
            q_group_block(q, lambda q_: sc_h1)
        po_h1 = pp.tile([H1, D], F32, tag="kp_ps", name="po_h1")
        epilogue_part(1, sc_h1, po_h1, H0, H1)

    nc.compile()
    return nc


_GRAPH_CACHE: dict[int, bass.Bass] = {}
_LAST_RESULTS = None


def _get_graph(KE: int) -> bass.Bass:
    if KE not in _GRAPH_CACHE:
        _GRAPH_CACHE[KE] = build_graph(KE)
    return _GRAPH_CACHE[KE]


def _sbuf_pack(mat_T):
    """[R*128, N] -> [128, R*N]: SBUF image with d-chunks along columns."""
    R = mat_T.shape[0] // 128
    return np.ascontiguousarray(
        mat_T.reshape(R, 128, -1).transpose(1, 0, 2).reshape(128, -1)
    )


def make_in_maps(queries, keys, values, Wq, Wk, wv, valid_lens, KE):
    wvwin = np.zeros((H, 128), BF16_NP)
    wvwin[:, 30] = wv.astype(BF16_NP)
    wvwin[:, 64 + 31] = wv.astype(BF16_NP)
    col = np.arange(KE)
    # k-chunk-major packing for kT: [128, sum_ci DC*w_ci]
    k_chunks = [(s, min(512, KE - s)) for s in range(0, KE, 512)]
    in_maps = []
    for c in range(N_CORES):
        b, qh = divmod(c, 2)
        mask_row = (col < int(valid_lens[b])).astype(np.float32)
        kT = keys[b, :KE, :].T.astype(BF16_NP)  # [D, KE]
        kT_packed = np.concatenate(
            [_sbuf_pack(kT[:, s : s + w]) for s, w in k_chunks], axis=1
        )
        in_maps.append(
            {
                "qT": _sbuf_pack(
                    queries[b, qh * QS : (qh + 1) * QS, :].T.astype(BF16_NP)
                ),
                "kT": np.ascontiguousarray(kT_packed),
                "v": _sbuf_pack(values[b].astype(BF16_NP)),
                "wq": _sbuf_pack(Wq.astype(BF16_NP)),
                "wk": _sbuf_pack(Wk.astype(BF16_NP)),
                "wvwin": wvwin,
                "mask": np.ascontiguousarray(
                    np.broadcast_to(mask_row, (96, KE)).astype(BF16_NP)
                ),
            }
        )
    return in_maps


def kernel(queries, keys, values, Wq, Wk, wv, valid_lens, **run_kwargs):
    queries = np.asarray(queries, np.float32)
    keys = np.asarray(keys, np.float32)
    values = np.asarray(values, np.float32)
    Wq = np.asarray(Wq, np.float32)
    Wk = np.asarray(Wk, np.float32)
    wv = np.asarray(wv, np.float32)
    valid_lens = np.asarray(valid_lens, np.int32)

    KE = int(-(-int(valid_lens.max()) // 128) * 128)
    KE = max(128, min(K, KE))

    nc = _get_graph(KE)
    in_maps = make_in_maps(queries, keys, values, Wq, Wk, wv, valid_lens, KE)
    res = run_bass_kernel_spmd(
        nc, in_maps, core_ids=list(range(N_CORES)), **run_kwargs
    )
    global _LAST_RESULTS
    _LAST_RESULTS = res
    out = np.empty((B, Q, D), np.float32)
    for c in range(N_CORES):
        b, qh = divmod(c, 2)
        out[b, qh * QS : (qh + 1) * QS, :] = res.results[c]["out"]
    return out
